# revision 1
# baseline (speedup 1.0000x reference)
"""DensityAwareChamferLoss Trainium2 kernel.

Strategy: 8 cores = (4 batches) x (2 NN directions). Each core runs an
identical SPMD program computing, for 8192 query points against 8192
candidate points, the argmin of squared euclidean distance:

  PE:  s = 2*q.c - |c|^2 at fp32-grade precision but bf16 matmul speed
       (1 cy/row): error-compensated bf16^3 decomposition packed along
       the contraction dim as ONE K=21 bf16 matmul per tile — product
       terms (qh,Ch)(qh,Cl)(ql,Ch)(ql,Cl)(qh,Cm)(qm,Ch) with C=2c plus
       three |c|^2 rows; exact bf16 products accumulate in fp32 PSUM,
       residual ~2^-24 (plain fp32 matmul is 4 cy/row = 874us/core;
       fp32r is tf32-grade and flips ~1.8% of argmins).
  ACT: d = |q|^2 - s  (scale=-1, per-partition bias), cast bf16 -> SBUF
  DVE: fused min-reduce over the [128, 8192] strip (tensor_scalar accum),
       then max_index to recover up to 8 positions matching the min.
       (A tensor_tensor_reduce fold variant — fold=True, ~574us modeled,
       CoreSim-exact — is present but disabled: its only HW attempt hit
       NRT_EXEC_UNIT_UNRECOVERABLE and could not be re-verified.)

Host: bf16 ties (~0.4% of rows) are resolved by recomputing that row's
distances in fp32 (reproduces the reference argmin: 0 flips measured in
numpy simulation and CoreSim); counts/weights/loss are O(N) numpy.

Engine budget per core (HW-calibrated cost model): PE ~250us, ACT
~580us, DVE ~690us (bound: max_index at 1x). With strip_bufs=4 /
small_bufs=12 the cross-tile pipeline hides everything but DVE:
~715us total — the config verified on silicon (PASS, rel err 7.2e-8).
"""

import sys

if "/opt/trn_rl_repo" not in sys.path:
    sys.path.insert(0, "/opt/trn_rl_repo")

import numpy as np

B = 4
N = 8192
QT = N // 128  # query tiles per core
N_CORES = 8

_CACHE = {}


def _build(mm_dtype="float32", do_accum=True, do_argidx=True, reps=1,
           strip_bufs=2, psum_bufs=2, small_bufs=4, kdim=4, fold=False):
    from contextlib import ExitStack

    import concourse.bacc as bacc
    import concourse.bass as bass
    import concourse.tile as tile
    from concourse import mybir

    f32 = mybir.dt.float32
    mmdt = getattr(mybir.dt, mm_dtype)
    bf16 = mybir.dt.bfloat16
    u32 = mybir.dt.uint32

    if kdim != 4:
        mmdt = bf16
    nc = bacc.Bacc("TRN2", target_bir_lowering=False, debug=False)
    qt4 = nc.dram_tensor("qt4", [kdim, N], mmdt, kind="ExternalInput")
    ct4 = nc.dram_tensor("ct4", [kdim, N], mmdt, kind="ExternalInput")
    qsq = nc.dram_tensor("qsq", [128, QT], f32, kind="ExternalInput")
    if do_argidx:
        out_idx = nc.dram_tensor("out_idx", [QT, 128, 8], u32, kind="ExternalOutput")
    else:
        out_min = nc.dram_tensor("out_min", [QT, 128, 8], f32, kind="ExternalOutput")

    with tile.TileContext(nc) as tc:
        with ExitStack() as ctx:
            const = ctx.enter_context(tc.tile_pool(name="const", bufs=1))
            strips = ctx.enter_context(tc.tile_pool(name="strip", bufs=strip_bufs))
            psum = ctx.enter_context(
                tc.tile_pool(name="psum", bufs=psum_bufs, space="PSUM"))
            small = ctx.enter_context(tc.tile_pool(name="small", bufs=small_bufs))

            qt4_s = const.tile([kdim, N], mmdt)
            nc.sync.dma_start(qt4_s[:], qt4.ap())
            ct4_s = const.tile([kdim, N], mmdt)
            nc.sync.dma_start(ct4_s[:], ct4.ap())
            qsq_s = const.tile([128, QT], f32)
            nc.sync.dma_start(qsq_s[:], qsq.ap())
            zeros8 = const.tile([128, 8], f32)
            nc.vector.memset(zeros8[:], 0.0)

            for t in [tt for _ in range(reps) for tt in range(QT)]:
                strip = strips.tile([128, N], bf16, tag="strip")
                for g in range(4):
                    ps = psum.tile([128, 2048], f32, tag="ps")
                    for j in range(4):
                        nc.tensor.matmul(
                            ps[:, j * 512 : (j + 1) * 512],
                            qt4_s[:, t * 128 : (t + 1) * 128],
                            ct4_s[:, g * 2048 + j * 512 : g * 2048 + (j + 1) * 512],
                            start=True,
                            stop=True,
                        )
                    # d = -s + |q|^2, cast to bf16
                    nc.scalar.activation(
                        strip[:, g * 2048 : (g + 1) * 2048],
                        ps[:],
                        mybir.ActivationFunctionType.Identity,
                        bias=qsq_s[:, t : t + 1],
                        scale=-1.0,
                    )
                if fold == "safe":
                    # same fold using only HW-verified encodings: plain TT min
                    # (bf16 2x) then the proven tensor_scalar accum on h
                    dmin = small.tile([128, 1], f32, tag="dmin")
                    h = small.tile([128, N // 2], bf16, tag="h")
                    nc.vector.tensor_tensor(
                        out=h[:],
                        in0=strip[:, : N // 2],
                        in1=strip[:, N // 2 :],
                        op=mybir.AluOpType.min,
                    )
                    nc.vector.tensor_scalar(
                        out=h[:],
                        in0=h[:],
                        scalar1=0.0,
                        scalar2=None,
                        op0=mybir.AluOpType.add,
                        op1=mybir.AluOpType.min,
                        accum_out=dmin[:],
                    )
                elif fold:
                    # fused: h = min(lo half, hi half) AND dmin = min(h)
                    dmin = small.tile([128, 1], f32, tag="dmin")
                    h = small.tile([128, N // 2], bf16, tag="h")
                    nc.vector.tensor_tensor_reduce(
                        out=h[:],
                        in0=strip[:, : N // 2],
                        in1=strip[:, N // 2 :],
                        scale=1.0,
                        scalar=3.0e38,
                        op0=mybir.AluOpType.min,
                        op1=mybir.AluOpType.min,
                        accum_out=dmin[:],
                    )
                elif do_accum:
                    # fused: rewrite strip in place (x+0) and min-reduce into dmin
                    dmin = small.tile([128, 1], f32, tag="dmin")
                    nc.vector.tensor_scalar(
                        out=strip[:],
                        in0=strip[:],
                        scalar1=0.0,
                        scalar2=None,
                        op0=mybir.AluOpType.add,
                        op1=mybir.AluOpType.min,
                        accum_out=dmin[:],
                    )
                if do_argidx:
                    # broadcast dmin to [128, 8] bf16 via ACT (scale=0, bias=dmin)
                    min8 = small.tile([128, 8], bf16, tag="min8")
                    nc.scalar.activation(
                        min8[:],
                        zeros8[:],
                        mybir.ActivationFunctionType.Identity,
                        bias=dmin[:],
                        scale=0.0,
                    )
                    idx8 = small.tile([128, 8], u32, tag="idx8")
                    nc.vector.max_index(idx8[:], min8[:],
                                        h[:] if fold else strip[:])
                    nc.sync.dma_start(out_idx.ap()[t], idx8[:])
                elif do_accum:
                    omin = small.tile([128, 8], f32, tag="omin")
                    nc.scalar.activation(
                        omin[:], zeros8[:],
                        mybir.ActivationFunctionType.Identity,
                        bias=dmin[:], scale=0.0,
                    )
                    nc.sync.dma_start(out_min.ap()[t], omin[:])
                else:
                    probe = small.tile([128, 8], f32, tag="omin")
                    sap = bass.AP(strip[:].tensor, strip[:].offset,
                                  [strip[:].ap[0], [1024, 8]])
                    nc.vector.tensor_copy(probe[:], sap)
                    nc.sync.dma_start(out_min.ap()[t], probe[:])

    nc.compile()
    return nc


def _prep_core_inputs(q, c):
    # q, c: [N, 3] float32
    qt4 = np.empty((4, N), np.float32)
    qt4[0:3] = q.T
    qt4[3] = 1.0
    ct4 = np.empty((4, N), np.float32)
    ct4[0:3] = 2.0 * c.T
    csq = np.sum(c.astype(np.float32) * c.astype(np.float32), axis=1)
    ct4[3] = -csq
    qsq_flat = np.sum(q.astype(np.float32) * q.astype(np.float32), axis=1)
    qsq = qsq_flat.reshape(QT, 128).T.copy()
    return {"qt4": qt4, "ct4": ct4, "qsq": qsq}


def _bf16_split3(x):
    # x (fp32) == hi + lo + mid to ~2^-24 rel; parts exactly bf16
    import ml_dtypes

    bf = ml_dtypes.bfloat16
    hi = x.astype(bf)
    r1 = (x - hi.astype(np.float32)).astype(np.float32)
    lo = r1.astype(bf)
    r2 = (r1 - lo.astype(np.float32)).astype(np.float32)
    mid = r2.astype(bf)
    return hi, lo, mid


def _prep_core_inputs_k21(q, c):
    """Error-compensated bf16^3 decomposition packed along K=21.

    s = sum_k lhsT[k]*rhs[k] = 2q.c - |c|^2 to ~2^-24 relative:
    product terms (qh,Ch),(qh,Cl),(ql,Ch),(ql,Cl),(qh,Cm),(qm,Ch) where
    C = 2c, plus (1,-csq_{h,l,m}). Each part is exactly bf16; PE computes
    exact bf16 x bf16 products accumulated in fp32 PSUM.
    """
    import ml_dtypes

    bf = ml_dtypes.bfloat16
    qh, ql, qm = _bf16_split3(np.ascontiguousarray(q.T, np.float32))  # [3, N]
    Ch, Cl, Cm = _bf16_split3(2.0 * np.ascontiguousarray(c.T, np.float32))
    csq = np.sum(c.astype(np.float32) * c.astype(np.float32), axis=1)
    sh, sl, sm = _bf16_split3(-csq)
    ones = np.ones((1, N), bf)
    qt = np.concatenate(
        [qh, qh, ql, ql, qh, qm, ones, ones, ones], axis=0
    ).astype(bf)
    ct = np.concatenate(
        [Ch, Cl, Ch, Cl, Cm, Ch, sh[None], sl[None], sm[None]], axis=0
    ).astype(bf)
    qsq_flat = np.sum(q.astype(np.float32) * q.astype(np.float32), axis=1)
    qsq = qsq_flat.reshape(QT, 128).T.copy()
    return {"qt4": qt, "ct4": ct, "qsq": qsq}


def _d_row_fp32(q_row, c_all):
    # reference-formula distances of one query row vs all candidates, fp32
    return (
        np.sum(q_row * q_row).astype(np.float32)
        + np.sum(c_all * c_all, axis=1)
        - 2.0 * (c_all @ q_row)
    ).astype(np.float32)


def _indices_from_out(idx8, q, c):
    # idx8: [QT, 128, 8] uint32 -> idx [N] with host tie fixup
    cand = idx8.reshape(N, 8)
    idx = cand[:, 0].astype(np.int64)
    ambiguous = np.where(cand[:, 1] != np.uint32(0xFFFFFFFF))[0]
    for r in ambiguous:
        d = _d_row_fp32(q[r], c)
        idx[r] = int(np.argmin(d))
    return idx


def _loss_one(q, c, idx):
    # mean(1 - exp(-d) * (1/(count+eps))) for one direction (frac terms = 1)
    d = np.sum((q - c[idx]) ** 2, axis=1).astype(np.float32)
    cnt = np.bincount(idx, minlength=N).astype(np.float32)
    w = np.float32(1.0) / (cnt[idx] + np.float32(1e-6))
    return np.mean(np.float32(1.0) - np.exp(-d) * w, dtype=np.float32)


def run_cores(in_maps, trace=False):
    from concourse.bass_utils import run_bass_kernel_spmd

    if "nc" not in _CACHE:
        # fold=True models ~574us and is CoreSim-exact, but it crashes the
        # exec unit on silicon (NRT_EXEC_UNIT_UNRECOVERABLE, reproduced 2/2
        # on a freshly-verified-healthy device) — the tensor_tensor_reduce
        # bf16-out+accum encoding is the suspect. Keep it disabled.
        _CACHE["nc"] = _build(kdim=21, strip_bufs=4, small_bufs=12)
    nc = _CACHE["nc"]
    res = run_bass_kernel_spmd(
        nc, in_maps, core_ids=list(range(N_CORES)), trace=trace
    )
    return res


def kernel(gts, preds):
    gts = np.ascontiguousarray(np.asarray(gts, dtype=np.float32))
    preds = np.ascontiguousarray(np.asarray(preds, dtype=np.float32))

    qc = []  # per-core (q, c)
    for core in range(N_CORES):
        b, direction = core >> 1, core & 1
        if direction == 0:
            qc.append((gts[b], preds[b]))
        else:
            qc.append((preds[b], gts[b]))

    in_maps = [_prep_core_inputs_k21(q, c) for (q, c) in qc]
    res = run_cores(in_maps)

    loss = np.zeros(B, np.float32)
    per_dir = {}
    for core in range(N_CORES):
        q, c = qc[core]
        idx = _indices_from_out(np.asarray(res.results[core]["out_idx"]), q, c)
        per_dir[core] = _loss_one(q, c, idx)
    for b in range(B):
        loss[b] = (per_dir[2 * b] + per_dir[2 * b + 1]) / np.float32(2.0)
    return loss


def _indices_from_out_fold(idx8, q, c):
    # idx8: [QT, 128, 8] positions in the folded half-strip; each expands to
    # {p, p+N/2}. Pick by exact fp32 reference-formula distance; full-row
    # fixup on exact ties or candidate-list overflow.
    H = N // 2
    cand_h = idx8.reshape(N, 8)
    valid = cand_h != np.uint32(0xFFFFFFFF)
    ch = np.where(valid, cand_h, 0).astype(np.int64)
    cands = np.concatenate([ch, ch + H], axis=1)  # [N, 16]
    vmask = np.concatenate([valid, valid], axis=1)
    qsq = np.sum(q.astype(np.float32) * q, axis=1).astype(np.float32)
    csq = np.sum(c.astype(np.float32) * c, axis=1).astype(np.float32)
    dots = np.einsum("rkd,rd->rk", c[cands], q.astype(np.float32),
                     dtype=np.float32).astype(np.float32)
    dc = (qsq[:, None] + csq[cands] - np.float32(2.0) * dots).astype(np.float32)
    dc[~vmask] = np.inf
    best = np.argmin(dc, axis=1)
    idx = cands[np.arange(N), best]
    dmin = dc[np.arange(N), best]
    n_min = (dc == dmin[:, None]).sum(1)
    fix = np.where((n_min > 1) | valid[:, 7])[0]
    for r in fix:
        idx[r] = int(np.argmin(_d_row_fp32(q[r], c)))
    return idx



# revision 5
# speedup vs baseline: 2.2492x; 2.2492x over previous
"""DensityAwareChamferLoss Trainium2 kernel.

Strategy: 8 cores = (4 batches) x (2 NN directions). Each core runs an
identical SPMD program over 8192 query points x 8192 candidates:

  PE:  d = |q|^2 + |c|^2 - 2 q.c at fp32-grade accuracy via an
       error-compensated bf16^3 decomposition packed along K=24 (six
       (q*,C*) product row-triples with C=-2c, three (1,csq*) rows,
       three (qsq*,1) rows), accumulated exactly in fp32 PSUM. Matmul
       cost is per output column, so the extra K rows are free.
  ACT: Copy (table-free) casts 4 of the 8 PSUM groups per query-tile to
       bf16 in SBUF (ACT is the cheapest PSUM-drain engine).
  DVE: grouped tensor_reduce (X-axis min over runs of 16) drains the
       other 4 PSUM groups directly to 64-wide bf16 minima -- fused
       drain+fold, no merge ops, no max_index.

The 8192 fp32/tile PSUM drain through ACT+DVE at ~1 elem/cycle is the
roofline here; engine budgets per tile: PE ~3.4us, ACT ~4.2us,
DVE ~4.8us -> ~64*5us ~ 320us/core vs 715us for the max_index baseline.

Host: per query row the bf16 row-min is found over the 4096 copied
values + 256 group minima; matching positions (group hits expand x16)
are re-evaluated in exact fp32 and the lowest-index argmin taken --
identical tie semantics to np.argmin since bf16 rounding is monotonic.
Counts/weights/loss are O(N) numpy.
"""

import sys

if "/opt/trn_rl_repo" not in sys.path:
    sys.path.insert(0, "/opt/trn_rl_repo")

import numpy as np

B = 4
N = 8192
QT = N // 128  # query tiles per core
K = 24
GRP = 16       # tensor_reduce group width
N_CORES = 8

# candidate-base offset of each output region (per 1024-wide psum group)
S_BASE = [0, 1024, 4096, 5120]      # ACT-copied groups
R_BASE = [2048, 3072, 6144, 7168]   # DVE-reduced groups

_CACHE = {}


def _build(strip_bufs=3, psum_bufs=4):
    from contextlib import ExitStack

    import concourse.bacc as bacc
    import concourse.bass as bass
    import concourse.tile as tile
    from concourse import mybir

    f32 = mybir.dt.float32
    bf16 = mybir.dt.bfloat16
    mn = mybir.AluOpType.min

    nc = bacc.Bacc("TRN2", target_bir_lowering=False, debug=False)
    qt = nc.dram_tensor("qt", [K, N], bf16, kind="ExternalInput")
    ct = nc.dram_tensor("ct", [K, N], bf16, kind="ExternalInput")
    out_s = nc.dram_tensor("out_s", [QT, 128, 4096], bf16, kind="ExternalOutput")
    out_r = nc.dram_tensor("out_r", [QT, 128, 256], bf16, kind="ExternalOutput")

    with tile.TileContext(nc) as tc:
        with ExitStack() as ctx:
            const = ctx.enter_context(tc.tile_pool(name="const", bufs=1))
            psum = ctx.enter_context(
                tc.tile_pool(name="psum", bufs=psum_bufs, space="PSUM"))
            strip = ctx.enter_context(tc.tile_pool(name="strip", bufs=strip_bufs))

            qt_s = const.tile([K, N], bf16)
            nc.sync.dma_start(qt_s[:], qt.ap())
            ct_s = const.tile([K, N], bf16)
            nc.sync.dma_start(ct_s[:], ct.ap())

            for t in range(QT):
                lhs = qt_s[:, t * 128 : (t + 1) * 128]
                s_tile = strip.tile([128, 4096], bf16, tag="s")
                r_tile = strip.tile([128, 256], bf16, tag="r")
                for half in range(2):
                    for k in range(2):  # ACT-destined groups
                        base = S_BASE[half * 2 + k]
                        pg = psum.tile([128, 1024], f32, tag="ps")
                        for j in range(2):
                            nc.tensor.matmul(
                                pg[:, j * 512 : (j + 1) * 512],
                                lhs,
                                ct_s[:, base + j * 512 : base + (j + 1) * 512],
                                start=True,
                                stop=True,
                            )
                        o = half * 2 + k
                        nc.scalar.copy(
                            s_tile[:, o * 1024 : (o + 1) * 1024], pg[:])
                    for k in range(2):  # DVE-destined groups
                        base = R_BASE[half * 2 + k]
                        pg = psum.tile([128, 1024], f32, tag="ps")
                        for j in range(2):
                            nc.tensor.matmul(
                                pg[:, j * 512 : (j + 1) * 512],
                                lhs,
                                ct_s[:, base + j * 512 : base + (j + 1) * 512],
                                start=True,
                                stop=True,
                            )
                        pv = pg[:]
                        ap3 = bass.AP(
                            pv.tensor, pv.offset,
                            [pv.ap[0], [GRP, 1024 // GRP], [1, GRP]])
                        o = half * 2 + k
                        nc.vector.tensor_reduce(
                            out=r_tile[:, o * 64 : (o + 1) * 64],
                            in_=ap3,
                            axis=mybir.AxisListType.X,
                            op=mn,
                        )
                nc.sync.dma_start(out_s.ap()[t], s_tile[:])
                nc.sync.dma_start(out_r.ap()[t], r_tile[:])

    nc.compile()
    return nc


def _bf16_split3(x):
    # x (fp32) == hi + lo + mid to ~2^-24 rel; parts exactly bf16
    import ml_dtypes

    bf = ml_dtypes.bfloat16
    hi = x.astype(bf)
    r1 = (x - hi.astype(np.float32)).astype(np.float32)
    lo = r1.astype(bf)
    r2 = (r1 - lo.astype(np.float32)).astype(np.float32)
    mid = r2.astype(bf)
    return hi, lo, mid


def _prep_core_inputs(q, c):
    """K=24 error-compensated bf16^3 decomposition of the full sq-distance.

    d = |q|^2 + |c|^2 - 2q.c = sum_k qt[k] * ct[k] to ~2^-24 relative:
    products (qh,Ch),(qh,Cl),(ql,Ch),(ql,Cl),(qh,Cm),(qm,Ch) with C=-2c,
    plus (1,csq_{h,l,m}) and (qsq_{h,l,m},1). Each part is exactly bf16;
    PE computes exact bf16 x bf16 products accumulated in fp32 PSUM.
    """
    import ml_dtypes

    bf = ml_dtypes.bfloat16
    qh, ql, qm = _bf16_split3(np.ascontiguousarray(q.T, np.float32))    # [3, N]
    Ch, Cl, Cm = _bf16_split3(np.ascontiguousarray(-2.0 * c.T, np.float32))
    csq = np.sum(c.astype(np.float32) * c.astype(np.float32), axis=1)
    qsq = np.sum(q.astype(np.float32) * q.astype(np.float32), axis=1)
    ch, cl, cm = _bf16_split3(csq)
    sh, sl, sm = _bf16_split3(qsq)
    ones = np.ones((1, N), bf)
    qtm = np.concatenate(
        [qh, qh, ql, ql, qh, qm, ones, ones, ones, sh[None], sl[None], sm[None]],
        axis=0,
    ).astype(bf)
    ctm = np.concatenate(
        [Ch, Cl, Ch, Cl, Cm, Ch, ch[None], cl[None], cm[None], ones, ones, ones],
        axis=0,
    ).astype(bf)
    return {"qt": qtm, "ct": ctm}


# column j of the [N, 4096+256] host matrix -> (cand base, span)
def _col_maps():
    sbase = np.empty(4096, np.int64)
    for k, b in enumerate(S_BASE):
        sbase[k * 1024 : (k + 1) * 1024] = b + np.arange(1024)
    rbase = np.empty(256, np.int64)
    for k, b in enumerate(R_BASE):
        rbase[k * 64 : (k + 1) * 64] = b + np.arange(64) * GRP
    return sbase, rbase


_SBASE_COLS, _RBASE_COLS = _col_maps()


def _indices_from_sr(s_out, r_out, q, c):
    """s_out: [QT,128,4096], r_out: [QT,128,256] bf16. Returns idx [N]
    matching np.argmin of the fp32 distance matrix (lowest index ties)."""
    S = np.asarray(s_out).reshape(N, 4096).astype(np.float32)
    R = np.asarray(r_out).reshape(N, 256).astype(np.float32)
    m = np.minimum(S.min(axis=1), R.min(axis=1))

    matchS = S == m[:, None]
    matchR = R == m[:, None]
    nS = matchS.sum(axis=1)
    nR = matchR.sum(axis=1)

    qf = q.astype(np.float32)
    cf = c.astype(np.float32)
    csq = np.sum(cf * cf, axis=1)
    qsq = np.sum(qf * qf, axis=1)

    idx = np.empty(N, np.int64)

    # case A: unique match, in the copied region -> position is exact
    rowsA = (nS == 1) & (nR == 0)
    idx[rowsA] = _SBASE_COLS[matchS[rowsA].argmax(axis=1)]

    # case B: unique match, one 16-wide reduced group -> exact eval of 16
    rowsB = np.flatnonzero((nS == 0) & (nR == 1))
    if len(rowsB):
        base = _RBASE_COLS[matchR[rowsB].argmax(axis=1)]
        cands = base[:, None] + np.arange(GRP)[None, :]          # [nB, GRP]
        dots = np.einsum("rkd,rd->rk", cf[cands], qf[rowsB]).astype(np.float32)
        dc = (qsq[rowsB, None] + csq[cands]
              - np.float32(2.0) * dots).astype(np.float32)
        best = dc.min(axis=1)
        sel = (dc == best[:, None]).argmax(axis=1)  # lowest index (ascending)
        idx[rowsB] = cands[np.arange(len(rowsB)), sel]

    # rare: bf16 ties across regions/groups -> expand everything
    for r in np.flatnonzero(~(rowsA | ((nS == 0) & (nR == 1)))):
        cands = _SBASE_COLS[np.flatnonzero(matchS[r])]
        rcols = np.flatnonzero(matchR[r])
        if len(rcols):
            expand = (_RBASE_COLS[rcols][:, None] + np.arange(GRP)[None, :]).ravel()
            cands = np.concatenate([cands, expand])
        dr = qsq[r] + csq[cands] - np.float32(2.0) * (cf[cands] @ qf[r])
        idx[r] = cands[np.flatnonzero(dr == dr.min())].min()
    return idx


def _loss_one(q, c, idx):
    # mean(1 - exp(-d) * (1/(count+eps))) for one direction (frac terms = 1)
    d = np.sum((q - c[idx]) ** 2, axis=1).astype(np.float32)
    cnt = np.bincount(idx, minlength=N).astype(np.float32)
    w = np.float32(1.0) / (cnt[idx] + np.float32(1e-6))
    return np.mean(np.float32(1.0) - np.exp(-d) * w, dtype=np.float32)


def run_cores(in_maps, trace=False):
    from concourse.bass_utils import run_bass_kernel_spmd

    if "nc" not in _CACHE:
        _CACHE["nc"] = _build()
    nc = _CACHE["nc"]
    res = run_bass_kernel_spmd(
        nc, in_maps, core_ids=list(range(N_CORES)), trace=trace
    )
    return res


def kernel(gts, preds):
    gts = np.ascontiguousarray(np.asarray(gts, dtype=np.float32))
    preds = np.ascontiguousarray(np.asarray(preds, dtype=np.float32))

    qc = []  # per-core (q, c)
    for core in range(N_CORES):
        b, direction = core >> 1, core & 1
        if direction == 0:
            qc.append((gts[b], preds[b]))
        else:
            qc.append((preds[b], gts[b]))

    in_maps = [_prep_core_inputs(q, c) for (q, c) in qc]
    res = run_cores(in_maps)

    loss = np.zeros(B, np.float32)
    per_dir = {}
    for core in range(N_CORES):
        q, c = qc[core]
        idx = _indices_from_sr(
            res.results[core]["out_s"], res.results[core]["out_r"], q, c)
        per_dir[core] = _loss_one(q, c, idx)
    for b in range(B):
        loss[b] = (per_dir[2 * b] + per_dir[2 * b + 1]) / np.float32(2.0)
    return loss


# revision 12
# speedup vs baseline: 10.4060x; 4.6264x over previous
"""DensityAwareChamferLoss Trainium2 kernel.

Strategy: 8 cores = (4 batches) x (2 NN directions). Each core runs an
identical SPMD program over 8192 query points x 8192 candidates:

  PE:  d = |q|^2 + |c|^2 - 2 q.c at fp32-grade accuracy via an
       error-compensated bf16^3 decomposition packed along K=24 (six
       (q*,C*) product row-triples with C=-2c, three (1,csq*) rows,
       three (qsq*,1) rows), accumulated exactly in fp32 PSUM. Matmul
       cost is per output column, so the extra K rows are free.
  ACT: Copy (table-free) casts 4 of the 8 PSUM groups per query-tile to
       bf16 in SBUF (ACT is the cheapest PSUM-drain engine).
  DVE: grouped tensor_reduce (X-axis min over runs of 16) drains the
       other 4 PSUM groups directly to 64-wide bf16 minima -- fused
       drain+fold, no merge ops, no max_index.

The 8192 fp32/tile PSUM drain through ACT+DVE at ~1 elem/cycle is the
roofline here; engine budgets per tile: PE ~3.4us, ACT ~4.2us,
DVE ~4.8us -> ~64*5us ~ 320us/core vs 715us for the max_index baseline.

Host: per query row the bf16 row-min is found over the 4096 copied
values + 256 group minima; matching positions (group hits expand x16)
are re-evaluated in exact fp32 and the lowest-index argmin taken --
identical tie semantics to np.argmin since bf16 rounding is monotonic.
Counts/weights/loss are O(N) numpy.
"""

import sys

if "/opt/trn_rl_repo" not in sys.path:
    sys.path.insert(0, "/opt/trn_rl_repo")

import numpy as np

B = 4
N = 8192
QT = N // 128  # query tiles per core
K = 24
GRP = 16       # tensor_reduce group width
N_CORES = 8

# candidate-base offset of each output region (per 1024-wide psum group)
S_BASE = [0, 1024, 4096, 5120]      # ACT-copied groups
R_BASE = [2048, 3072, 6144, 7168]   # DVE-reduced groups (last one 768 wide)
S_TAIL = 7936                       # last 256 cols of group 7: ACT-copied
R_COLS = 3 * 64 + 48                # 240 reduced output columns

_CACHE = {}


def _build(strip_bufs=3, psum_bufs=4):
    from contextlib import ExitStack

    import concourse.bacc as bacc
    import concourse.bass as bass
    import concourse.tile as tile
    from concourse import mybir

    f32 = mybir.dt.float32
    bf16 = mybir.dt.bfloat16
    mn = mybir.AluOpType.min

    nc = bacc.Bacc("TRN2", target_bir_lowering=False, debug=False)
    qt = nc.dram_tensor("qt", [K, N], bf16, kind="ExternalInput")
    ct = nc.dram_tensor("ct", [K, N], bf16, kind="ExternalInput")
    out_s = nc.dram_tensor("out_s", [QT, 128, 4352], bf16, kind="ExternalOutput")
    out_r = nc.dram_tensor("out_r", [QT, 128, R_COLS], bf16, kind="ExternalOutput")

    with tile.TileContext(nc) as tc:
        with ExitStack() as ctx:
            const = ctx.enter_context(tc.tile_pool(name="const", bufs=1))
            psum = ctx.enter_context(
                tc.tile_pool(name="psum", bufs=psum_bufs, space="PSUM"))
            strip = ctx.enter_context(tc.tile_pool(name="strip", bufs=strip_bufs))

            qt_s = const.tile([K, N], bf16)
            nc.sync.dma_start(qt_s[:], qt.ap())
            ct_s = const.tile([K, N], bf16)
            nc.sync.dma_start(ct_s[:], ct.ap())

            for t in range(QT):
                lhs = qt_s[:, t * 128 : (t + 1) * 128]
                s_tile = strip.tile([128, 4352], bf16, tag="s")
                r_tile = strip.tile([128, R_COLS], bf16, tag="r")
                for half in range(2):
                    for k in range(2):  # ACT-destined groups
                        base = S_BASE[half * 2 + k]
                        pg = psum.tile([128, 1024], f32, tag="ps")
                        for j in range(2):
                            nc.tensor.matmul(
                                pg[:, j * 512 : (j + 1) * 512],
                                lhs,
                                ct_s[:, base + j * 512 : base + (j + 1) * 512],
                                start=True,
                                stop=True,
                            )
                        o = half * 2 + k
                        nc.scalar.copy(
                            s_tile[:, o * 1024 : (o + 1) * 1024], pg[:])
                    for k in range(2):  # DVE-destined groups
                        base = R_BASE[half * 2 + k]
                        pg = psum.tile([128, 1024], f32, tag="ps")
                        for j in range(2):
                            nc.tensor.matmul(
                                pg[:, j * 512 : (j + 1) * 512],
                                lhs,
                                ct_s[:, base + j * 512 : base + (j + 1) * 512],
                                start=True,
                                stop=True,
                            )
                        o = half * 2 + k
                        last = o == 3
                        # last group: reduce 768 cols; ACT copies the tail 256
                        width = 768 if last else 1024
                        pv = pg[:, :width]
                        ap3 = bass.AP(
                            pv.tensor, pv.offset,
                            [pv.ap[0], [GRP, width // GRP], [1, GRP]])
                        nc.vector.tensor_reduce(
                            out=r_tile[:, o * 64 : o * 64 + width // GRP],
                            in_=ap3,
                            axis=mybir.AxisListType.X,
                            op=mn,
                        )
                        if last:
                            nc.scalar.copy(s_tile[:, 4096:], pg[:, 768:])
                nc.sync.dma_start(out_s.ap()[t], s_tile[:])
                nc.sync.dma_start(out_r.ap()[t], r_tile[:])

    nc.compile()
    return nc


def _bf16_split3(x):
    # x (fp32) == hi + lo + mid to ~2^-24 rel; parts exactly bf16
    import ml_dtypes

    bf = ml_dtypes.bfloat16
    hi = x.astype(bf)
    r1 = (x - hi.astype(np.float32)).astype(np.float32)
    lo = r1.astype(bf)
    r2 = (r1 - lo.astype(np.float32)).astype(np.float32)
    mid = r2.astype(bf)
    return hi, lo, mid


def _prep_core_inputs(q, c):
    """K=24 error-compensated bf16^3 decomposition of the full sq-distance.

    d = |q|^2 + |c|^2 - 2q.c = sum_k qt[k] * ct[k] to ~2^-24 relative:
    products (qh,Ch),(qh,Cl),(ql,Ch),(ql,Cl),(qh,Cm),(qm,Ch) with C=-2c,
    plus (1,csq_{h,l,m}) and (qsq_{h,l,m},1). Each part is exactly bf16;
    PE computes exact bf16 x bf16 products accumulated in fp32 PSUM.
    """
    import ml_dtypes

    bf = ml_dtypes.bfloat16
    qh, ql, qm = _bf16_split3(np.ascontiguousarray(q.T, np.float32))    # [3, N]
    Ch, Cl, Cm = _bf16_split3(np.ascontiguousarray(-2.0 * c.T, np.float32))
    csq = np.sum(c.astype(np.float32) * c.astype(np.float32), axis=1)
    qsq = np.sum(q.astype(np.float32) * q.astype(np.float32), axis=1)
    ch, cl, cm = _bf16_split3(csq)
    sh, sl, sm = _bf16_split3(qsq)
    ones = np.ones((1, N), bf)
    qtm = np.concatenate(
        [qh, qh, ql, ql, qh, qm, ones, ones, ones, sh[None], sl[None], sm[None]],
        axis=0,
    ).astype(bf)
    ctm = np.concatenate(
        [Ch, Cl, Ch, Cl, Cm, Ch, ch[None], cl[None], cm[None], ones, ones, ones],
        axis=0,
    ).astype(bf)
    return {"qt": qtm, "ct": ctm}


def _col_maps():
    sbase = np.empty(4352, np.int64)
    for k, b in enumerate(S_BASE):
        sbase[k * 1024 : (k + 1) * 1024] = b + np.arange(1024)
    sbase[4096:] = S_TAIL + np.arange(256)
    rbase = np.empty(R_COLS, np.int64)
    for k, b in enumerate(R_BASE[:3]):
        rbase[k * 64 : (k + 1) * 64] = b + np.arange(64) * GRP
    rbase[192:240] = R_BASE[3] + np.arange(48) * GRP
    return sbase, rbase


_SBASE_COLS, _RBASE_COLS = _col_maps()


def _indices_from_sr(s_out, r_out, q, c):
    """s_out: [QT,128,4352] bf16, r_out: [QT,128,R_COLS] bf16. Returns
    idx [N] matching np.argmin of the fp32 distance matrix (lowest index
    on ties)."""
    S = np.asarray(s_out).reshape(N, 4352).astype(np.float32)
    R = np.asarray(r_out).reshape(N, R_COLS).astype(np.float32)
    m = np.minimum(S.min(axis=1), R.min(axis=1))

    scols = _SBASE_COLS
    matchS = S == m[:, None]
    matchR = R == m[:, None]
    nS = matchS.sum(axis=1)
    nR = matchR.sum(axis=1)

    qf = q.astype(np.float32)
    cf = c.astype(np.float32)
    csq = np.sum(cf * cf, axis=1)
    qsq = np.sum(qf * qf, axis=1)

    idx = np.empty(N, np.int64)

    # case A: unique match, in the copied region -> position is exact
    rowsA = (nS == 1) & (nR == 0)
    idx[rowsA] = scols[matchS[rowsA].argmax(axis=1)]

    # case B: unique match, one 16-wide reduced group -> exact eval of 16
    rowsB = np.flatnonzero((nS == 0) & (nR == 1))
    if len(rowsB):
        base = _RBASE_COLS[matchR[rowsB].argmax(axis=1)]
        cands = base[:, None] + np.arange(GRP)[None, :]          # [nB, GRP]
        dots = np.einsum("rkd,rd->rk", cf[cands], qf[rowsB]).astype(np.float32)
        dc = (qsq[rowsB, None] + csq[cands]
              - np.float32(2.0) * dots).astype(np.float32)
        best = dc.min(axis=1)
        sel = (dc == best[:, None]).argmax(axis=1)  # lowest index (ascending)
        idx[rowsB] = cands[np.arange(len(rowsB)), sel]

    # rare: bf16 ties across regions/groups -> expand everything
    for r in np.flatnonzero(~(rowsA | ((nS == 0) & (nR == 1)))):
        cands = scols[np.flatnonzero(matchS[r])]
        rcols = np.flatnonzero(matchR[r])
        if len(rcols):
            expand = (_RBASE_COLS[rcols][:, None] + np.arange(GRP)[None, :]).ravel()
            cands = np.concatenate([cands, expand])
        dr = qsq[r] + csq[cands] - np.float32(2.0) * (cf[cands] @ qf[r])
        idx[r] = cands[np.flatnonzero(dr == dr.min())].min()
    return idx


def _loss_one(q, c, idx):
    # mean(1 - exp(-d) * (1/(count+eps))) for one direction (frac terms = 1)
    d = np.sum((q - c[idx]) ** 2, axis=1).astype(np.float32)
    cnt = np.bincount(idx, minlength=N).astype(np.float32)
    w = np.float32(1.0) / (cnt[idx] + np.float32(1e-6))
    return np.mean(np.float32(1.0) - np.exp(-d) * w, dtype=np.float32)


def run_cores(in_maps, trace=False):
    from concourse.bass_utils import run_bass_kernel_spmd

    if "nc" not in _CACHE:
        _CACHE["nc"] = _build()
    nc = _CACHE["nc"]
    res = run_bass_kernel_spmd(
        nc, in_maps, core_ids=list(range(N_CORES)), trace=trace
    )
    return res


def kernel(gts, preds):
    gts = np.ascontiguousarray(np.asarray(gts, dtype=np.float32))
    preds = np.ascontiguousarray(np.asarray(preds, dtype=np.float32))

    qc = []  # per-core (q, c)
    for core in range(N_CORES):
        b, direction = core >> 1, core & 1
        if direction == 0:
            qc.append((gts[b], preds[b]))
        else:
            qc.append((preds[b], gts[b]))

    in_maps = [_prep_core_inputs(q, c) for (q, c) in qc]
    res = run_cores(in_maps)

    loss = np.zeros(B, np.float32)
    per_dir = {}
    for core in range(N_CORES):
        q, c = qc[core]
        idx = _indices_from_sr(
            res.results[core]["out_s"], res.results[core]["out_r"], q, c)
        per_dir[core] = _loss_one(q, c, idx)
    for b in range(B):
        loss[b] = (per_dir[2 * b] + per_dir[2 * b + 1]) / np.float32(2.0)
    return loss
